# revision 78
# baseline (speedup 1.0000x reference)
"""CQAttention (context-query attention) Trainium2 kernel, v2.

Problem (per batch b of 16):
    S  = (C@w1)[:,None] + (Q@w2)[None,:] + (C*w3)@Q^T          [Lc, Lq]
    S1 = softmax_j(S masked by qmask), S2 = softmax_i(S masked by cmask)
    A  = S1@Q ;  Z = S2^T@C ;  Bm = S1@Z
    out = [C, A, C*A, C*Bm] @ out_w^T + out_b                  [Lc, d]
with B=16, Lc=1024, Lq=512, d=512, fp32.

Sharding: data-parallel over batch, 2 batches per NeuronCore, no
collectives.

Key optimizations over the v1 kernel:
- qmask compaction (host-side gather): every use of the Lq axis only
  involves positions with qmask==1 (masked j contribute exactly 0 to
  S1 rows, to A/Bm, and Z[j] is multiplied by S1[:,j]==0), so Q is
  compacted to its unmasked rows and padded to LQC (multiple of 32,
  >= max count over batches). Padded columns get a -1e4 additive bias
  so exp underflows to exactly 0. This shrinks all Lq-dim compute by
  LQC/512 and drops the qmask entirely.
- Logits are computed ONCE (natural [Lc, LQC] layout); the transposed
  layout needed by the A/Bm contractions comes from PE transposes
  ([128,128] blocks through PSUM), which is ~3x cheaper than a second
  logit pass.
- The rank-1 logit terms: c1=C@w1 is produced directly as per-i-tile
  PSUM columns (ap=1 matmuls, ~free) and folded into the Exp
  activation as a per-partition bias; q2+padbias is added by a single
  K=2 ones-matmul accumulating into the logit PSUM group.
- Row sums (S1 denominators) ride the Exp eviction via ACT accum_out;
  the S1 normalization is a per-partition DVE scale producing EXPN,
  applied before the transposes, so A/Bm need no post-scaling.
- Column sums (S2 denominators) are ap=1 matmuls riding the Z
  accumulation (rhs = cmask column); the S2 normalization is folded
  into Z's PSUM->SBUF eviction as a per-partition ACT scale.
- cmask is folded into C on the host (binary select) for Z's rhs.
- The final linear is computed transposed (out^T = OW^T-tiles @
  out4^T); the host transposes the [d, Lc] result back.
- All matmul operands are float32r (full PE rate at N>=256).
- split_multi_waits works around this container's walrus, which
  rejects any instruction carrying more than one sync wait.
"""

import numpy as np

import concourse.bass as bass
import concourse.mybir as mybir
import concourse.tile as tile
from concourse.bass_utils import run_bass_kernel_spmd

F32 = mybir.dt.float32
F32R = mybir.dt.float32r
BF16 = mybir.dt.bfloat16
AF = mybir.ActivationFunctionType

B, LC, LQ, D = 16, 1024, 512, 512
NCORES = 8
BPC = B // NCORES  # batches per core
I_T, K_T = LC // 128, D // 128  # 8, 4
F_T = 4 * D // 128  # 16 feature tiles of out4
MASK_BIAS = 1.0e4  # exp(x - 1e4) == 0.0 exactly in fp32 for |x| ~ O(10)

SECTIONS = []
DEBUG_TAPS = False


def _mark(nc, label):
    SECTIONS.append((label, int(nc.get_next_instruction_name().split("-")[1])))


def split_multi_waits(nc):
    """This walrus build allows at most one sync wait per instruction;
    hoist extras onto standalone EventSemaphore (wait) instructions."""
    for f in nc.m.functions:
        for blk in f.blocks:
            new = []
            changed = False
            for inst in blk.instructions:
                si = inst.sync_info
                waits = list(si.on_wait) if si is not None else []
                if len(waits) > 1:
                    changed = True
                    for k, w in enumerate(waits[:-1]):
                        ev = mybir.InstEventSemaphore(
                            name=f"{inst.name}-sw{k}", ins=[], outs=[]
                        )
                        ev.engine = inst.engine
                        ev.sync_info = mybir.SyncInfo(on_wait=[w], on_update=[])
                        new.append(ev)
                    si.on_wait = [waits[-1]]
                    inst.sync_info = si
                new.append(inst)
            if changed:
                blk.instructions = new


def _jtiles(lqc):
    """[(j0, width)] partition tiles of the compacted Lq axis."""
    out = []
    j0 = 0
    while j0 < lqc:
        out.append((j0, min(128, lqc - j0)))
        j0 += 128
    return out


def _emit_front(nc, pools, consts, dram, b, lqc, dma_inputs=True):
    """Input DMAs + qw3t + q2/augr + (c1 is emitted inside the S loop)."""
    (sb, small, psum, rowps) = pools
    _mark(nc, f"b{b}.front")
    jt = _jtiles(lqc)

    # ---- critical input DMAs: two independent overhead rails ----
    # sync -> HWDGE (625 ns/dma), gpsimd -> Pool SWDGE (~1040 ns/dma); the
    # transfers themselves serialize on the shared DMA engines, so keep
    # transfers large and split by k parity so both rails stream columns.
    # Batch 0's inputs ride the sync/HWDGE rail in consumption order (qt,
    # ctq quarters, cmn, qn).  Batch 1's ride the pool/SWDGE rail with ctq
    # FIRST: its buffer-reuse wait (ctq bufs=4, freed by b0's out stage)
    # holds the whole b1 group at the pool queue head until ~b0's back
    # phase, which keeps b1 transfers from stealing DMA bandwidth that
    # b0's cmn/qn/ow stream needs.
    eng = nc.sync if b == 0 else nc.gpsimd
    augr = small.tile([2, lqc], F32R, tag="augr", bufs=2, name=f"augr_{b}")
    cmcol = small.tile([128, 2 * I_T], BF16, tag="cmcol", bufs=2, name=f"cmcol_{b}")
    # ct: 4 host-packed column-quarter blocks: ctq[q][:, k*256+c]
    # = C^T[k*128+p, q*256+c]. One DMA delivers a full quarter for ALL k,
    # matching the S loop's column-major consumption.
    ctq = []
    for q in range(4):
        t = sb.tile([128, LC], F32R, tag="ct", bufs=4, name=f"ctq{q}_{b}")
        ctq.append(t)
    qt_all = sb.tile([128, K_T * lqc], F32R, tag="qt", bufs=2, name=f"qt_{b}")
    if b == 0:
        eng.dma_start(out=qt_all[:], in_=dram["q_t"].ap()[b])
    for q in range(4):
        for h in range(2):
            eng.dma_start(out=ctq[q][:, h * 512:(h + 1) * 512],
                          in_=dram["c_t"].ap()[b, 0 + q, :, h * 512:(h + 1) * 512])
    if b != 0:
        eng.dma_start(out=qt_all[:], in_=dram["q_t"].ap()[b])

    def ctsl(k, c0, w):
        """AP for C^T[k*128:(k+1)*128, c0:c0+w] in the quarter-packed tiles;
        (c0, w) must lie within one 256-column quarter."""
        q, off = divmod(c0, 256)
        return ctq[q][:, k * 256 + off:k * 256 + off + w]
    cmn = []
    for i in range(I_T):
        t = sb.tile([128, D], BF16, tag="cmn", bufs=8, name=f"cmn{i}_{b}")
        eng.dma_start(out=t[:], in_=dram["cm_nat"].ap()[b, i * 128:(i + 1) * 128, :])
        cmn.append(t)
    qn = []
    for ji, (j0, jw) in enumerate(jt):
        t = sb.tile([128, D], F32R, tag="qn", bufs=len(jt), name=f"qn{ji}_{b}")
        eng.dma_start(out=t[:jw, :], in_=dram["q_nat"].ap()[b, j0:j0 + jw, :])
        qn.append(t)
    # pad-bias + cmask columns: tiny, always on the pool rail
    nc.gpsimd.dma_start(out=augr[1:2, :], in_=dram["pb"].ap()[b])
    nc.gpsimd.dma_start(out=cmcol[:], in_=dram["cm_col"].ap()[b])
    # ---- qw3t = Q^T * w3 (per-partition scale) ----
    (ones2, w1c, w2cb, identb, w3c, ow, obc) = consts
    qt = [qt_all[:, k * lqc:(k + 1) * lqc] for k in range(K_T)]
    qw3t = []
    for k in range(K_T):
        t = sb.tile([128, lqc], F32R, tag="qw3t", bufs=4, name=f"qw3t{k}_{b}")
        nc.vector.tensor_scalar_mul(t[:], qt[k], w3c[:, k:k + 1])
        qw3t.append(t)

    # ---- q2 = Q@w2 row; augr row0 <- q2 (row1 is the DMA'd pad bias) ----
    q2_t = psum.tile([128, 512], F32, tag="mmps", name=f"q2ps_{b}")
    q2_ps = q2_t[0:1, 0:lqc]
    for k in range(K_T):
        nc.tensor.matmul(q2_ps, w2cb[:, k:k + 1], qt[k],
                         start=(k == 0), stop=(k == K_T - 1))
    nc.scalar.copy(augr[0:1, :], q2_ps)

    return dict(qt=qt, ctsl=ctsl, cmn=cmn, qn=qn, cmcol=cmcol, augr=augr,
                qw3t=qw3t, w2cb=w2cb, identb=identb)


def _emit_mid(nc, pools, consts, dram, b, lqc, fr):
    """S logits + exp + rowsum-normalize + transposes + Z/colsum."""
    (sb, small, psum, rowps) = pools
    (ones2, w1c, w2cb, identb, w3c, ow, obc) = consts
    ctsl, qw3t, augr, cmcol, cmn = fr["ctsl"], fr["qw3t"], fr["augr"], fr["cmcol"], fr["cmn"]
    identb = fr["identb"]
    jt = _jtiles(lqc)

    _mark(nc, f"b{b}.s")
    # ---- c1 columns + S + exp(+rowsum accum) + S1-normalize, per i ----
    c1ps_t = psum.tile([128, 512], F32, tag="mmps", name=f"c1ps_{b}")
    c1ps = c1ps_t[:, 0:2 * I_T]
    c1col = small.tile([128, I_T], F32, tag="c1col", bufs=2, name=f"c1col_{b}")
    rs = small.tile([128, I_T], F32, tag="rs", bufs=2, name=f"rs_{b}")
    irs = small.tile([128, I_T], F32, tag="irs", bufs=2, name=f"irs_{b}")
    exp = []
    expn = []
    expb = []
    for i in range(I_T):
        # fp32r matmuls need output free >= 2: w1c holds duplicated column
        # pairs, so each (i, k) product lands as a [128, 2] pair; col 2i is c1.
        for k in range(K_T):
            nc.tensor.matmul(c1ps[:, 2 * i:2 * i + 2], ctsl(k, i * 128, 128),
                             w1c[:, 2 * k:2 * k + 2],
                             start=(k == 0 and i == 0),
                             stop=(k == K_T - 1 and i == I_T - 1))
        nc.vector.tensor_copy(c1col[:, i:i + 1], c1ps[:, 2 * i:2 * i + 1])
        s_t = psum.tile([128, 512], F32, tag="mmps", name=f"sps{i}_{b}")
        s_ps = s_t[:, 0:lqc]
        for k in range(K_T):
            nc.tensor.matmul(s_ps[:], ctsl(k, i * 128, 128), qw3t[k][:],
                             start=(k == 0), stop=False)
        nc.tensor.matmul(s_ps[:], ones2[:, :], augr[:, :], start=False, stop=True)
        e = sb.tile([128, lqc], F32R, tag="exp", bufs=8, name=f"exp{i}_{b}")
        nc.scalar.activation(e[:], s_ps[:], AF.Exp,
                             bias=c1col[:, i:i + 1], scale=1.0,
                             accum_out=rs[:, i:i + 1])
        exp.append(e)
        with nc.allow_low_precision(reason="softmax denominator reciprocal"):
            nc.vector.reciprocal(irs[:, i:i + 1], rs[:, i:i + 1])
        en = sb.tile([128, lqc], F32R, tag="expn", bufs=8, name=f"expn{i}_{b}")
        nc.vector.tensor_scalar_mul(en[:], e[:], irs[:, i:i + 1])
        expn.append(en)
        eb = sb.tile([128, lqc], BF16, tag="expb", bufs=8, name=f"expb{i}_{b}")
        nc.vector.tensor_copy(eb[:], e[:])
        expb.append(eb)

    _mark(nc, f"b{b}.tz")
    # ---- Z + colsum + transposes, interleaved by i so the PE rides the
    # cmn DMA arrivals (one cmn tile unblocks 3 z matmuls + 3 transposes) ----
    njt = len(jt)
    expt = [sb.tile([128, LC], F32R, tag="expt", bufs=njt, name=f"expt{ji}_{b}")
            for ji in range(njt)]
    z_ps = [psum.tile([128, D], F32, tag="mmps", name=f"zps{ji}_{b}")
            for ji in range(njt)]
    # each j's colsum pair gets its own 512B psum region: concurrent
    # accumulation groups may not share a zeroing granule
    zcs_t = psum.tile([128, 512], F32, tag="mmps", name=f"zcs_{b}")
    zinv = small.tile([128, njt], F32, tag="zinv", bufs=2, name=f"zinv_{b}")
    tp = [None] * njt
    for i in range(I_T):
        if i % 4 == 0:
            for ji, (j0, jw) in enumerate(jt):
                if i == 4:
                    nc.scalar.copy(expt[ji][:jw, 0:512], tp[ji][:jw, :])
                tp[ji] = psum.tile([128, 512], F32R, tag="mmps",
                                   name=f"tp{ji}_{i // 4}_{b}")
        for ji, (j0, jw) in enumerate(jt):
            nc.tensor.matmul(z_ps[ji][:jw, :], expb[i][:, j0:j0 + jw], cmn[i][:],
                             start=(i == 0), stop=(i == I_T - 1))
        for ji, (j0, jw) in enumerate(jt):
            # one group for the whole bank: a start lazily zeroes the
            # entire 2KB zero region, so only the very first matmul starts
            nc.tensor.matmul(zcs_t[:jw, 128 * ji:128 * ji + 2], expb[i][:, j0:j0 + jw],
                             cmcol[:, 2 * i:2 * i + 2],
                             start=(i == 0 and ji == 0),
                             stop=(i == I_T - 1 and ji == len(jt) - 1))
        for ji, (j0, jw) in enumerate(jt):
            nc.tensor.transpose(tp[ji][:jw, (i % 4) * 128:(i % 4 + 1) * 128],
                                expn[i][:, j0:j0 + jw], identb[:])
    z = []
    for ji, (j0, jw) in enumerate(jt):
        nc.scalar.copy(expt[ji][:jw, 512:1024], tp[ji][:jw, :])
        with nc.allow_low_precision(reason="softmax denominator reciprocal"):
            nc.vector.tensor_scalar_add(zinv[:jw, ji:ji + 1],
                                        zcs_t[:jw, 128 * ji:128 * ji + 1], 1e-30)
            nc.vector.reciprocal(zinv[:jw, ji:ji + 1], zinv[:jw, ji:ji + 1])
        zt = sb.tile([128, D], F32R, tag="z", bufs=njt, name=f"z{ji}_{b}")
        nc.scalar.mul(zt[:jw, :], z_ps[ji][:jw, :], zinv[:jw, ji:ji + 1])
        z.append(zt)

    fr["expt"] = expt
    fr["z"] = z
    if DEBUG_TAPS and b == 0:
        nc.sync.dma_start(out=dram["dbg_augr"].ap(), in_=augr[:])
        nc.sync.dma_start(out=dram["dbg_rs"].ap(), in_=rs[:])
        nc.sync.dma_start(out=dram["dbg_irs"].ap(), in_=irs[:])
        nc.sync.dma_start(out=dram["dbg_c1"].ap(), in_=c1col[:])
        nc.sync.dma_start(out=dram["dbg_zinv"].ap(), in_=zinv[:])
        nc.sync.dma_start(out=dram["dbg_exp0"].ap(), in_=exp[0][:])
        nc.sync.dma_start(out=dram["dbg_expt0"].ap(), in_=expt[0][:])
        nc.sync.dma_start(out=dram["dbg_z0"].ap(), in_=z[0][:])


def _emit_back(nc, pools, consts, dram, b, lqc, fr):
    """A/Bm (both n-chunks), then the out matmuls + stores."""
    (sb, small, psum, rowps) = pools
    (ones2, w1c, w2cb, identb, w3c, ow, obc) = consts
    ctsl, qn, expt, z = fr["ctsl"], fr["qn"], fr["expt"], fr["z"]
    jt = _jtiles(lqc)

    at_n = [[None] * K_T for _ in range(2)]
    cat_n = [[None] * K_T for _ in range(2)]
    cbt_n = [[None] * K_T for _ in range(2)]
    _mark(nc, f"b{b}.ab")
    for n in range(2):
        sl = slice(n * 512, (n + 1) * 512)
        for m in range(K_T):
            msl = slice(m * 128, (m + 1) * 128)
            a_ps = psum.tile([128, 512], F32, tag="mmps", name=f"aps{n}_{m}_{b}")
            for ji, (j0, jw) in enumerate(jt):
                nc.tensor.matmul(a_ps[:], qn[ji][:jw, msl], expt[ji][:jw, sl],
                                 start=(ji == 0), stop=(ji == len(jt) - 1))
            at = sb.tile([128, 512], F32R, tag="at", bufs=8, name=f"at{m}_{n}_{b}")
            nc.scalar.copy(at[:], a_ps[:])
            at_n[n][m] = at
            b_ps = psum.tile([128, 512], F32, tag="mmps", name=f"bps{n}_{m}_{b}")
            for ji, (j0, jw) in enumerate(jt):
                nc.tensor.matmul(b_ps[:], z[ji][:jw, msl], expt[ji][:jw, sl],
                                 start=(ji == 0), stop=(ji == len(jt) - 1))
            cbt = sb.tile([128, 512], F32R, tag="cbt", bufs=8, name=f"cbt{m}_{n}_{b}")
            nc.scalar.copy(cbt[:], b_ps[:])
            cbt_n[n][m] = cbt
            cat = sb.tile([128, 512], F32R, tag="cat", bufs=8, name=f"cat{m}_{n}_{b}")
            for h in range(2):
                hs = slice(h * 256, (h + 1) * 256)
                nc.vector.tensor_mul(cat[:, hs], ctsl(m, n * 512 + h * 256, 256),
                                     at[:, hs])
                nc.vector.tensor_mul(cbt[:, hs], ctsl(m, n * 512 + h * 256, 256),
                                     cbt[:, hs])
            cat_n[n][m] = cat

    for n in range(2):
        sl = slice(n * 512, (n + 1) * 512)
        _mark(nc, f"b{b}.out{n}")
        for m in range(K_T):
            o_ps = psum.tile([128, 512], F32, tag="mmps", name=f"ops{n}_{m}_{b}")
            for f in range(F_T):
                g, k = f // 4, f % 4
                lhs = ow[f][:, m * 128:(m + 1) * 128]
                if g == 0:
                    # ct is quarter-packed: the C-group rhs splits in half
                    for h in range(2):
                        nc.tensor.matmul(o_ps[:, h * 256:(h + 1) * 256], lhs,
                                         ctsl(k, n * 512 + h * 256, 256),
                                         start=(f == 0 and h == 0), stop=False)
                    continue
                elif g == 1:
                    rhs = at_n[n][k][:]
                elif g == 2:
                    rhs = cat_n[n][k][:]
                else:
                    rhs = cbt_n[n][k][:]
                nc.tensor.matmul(o_ps[:], lhs, rhs,
                                 start=(f == 0), stop=(f == F_T - 1))
            ot = sb.tile([128, 512], F32, tag="ot", bufs=4, name=f"ot{m}_{n}_{b}")
            last = (n == 1 and m == K_T - 1)
            for h in range(2) if last else (0,):
                hs = slice(h * 256, (h + 1) * 256) if last else slice(0, 512)
                nc.scalar.activation(ot[:, hs], o_ps[:, hs], AF.Identity,
                                     bias=obc[:, m:m + 1], scale=1.0)
                nc.sync.dma_start(
                    out=dram["out_t"].ap()[b, m * 128:(m + 1) * 128,
                                           n * 512 + hs.start:n * 512 + hs.stop],
                    in_=ot[:, hs])


def build(lqc):
    nc = bass.Bass("TRN2", target_bir_lowering=False, debug=False,
                   num_devices=NCORES)
    dram = {}
    dram["c_t"] = nc.dram_tensor("c_t", [BPC, 4, 128, LC], F32R, kind="ExternalInput")
    dram["cm_nat"] = nc.dram_tensor("cm_nat", [BPC, LC, D], BF16, kind="ExternalInput")
    dram["cm_col"] = nc.dram_tensor("cm_col", [BPC, 128, 2 * I_T], BF16, kind="ExternalInput")
    dram["q_t"] = nc.dram_tensor("q_t", [BPC, 128, K_T * lqc], F32R, kind="ExternalInput")
    dram["q_nat"] = nc.dram_tensor("q_nat", [BPC, lqc, D], F32R, kind="ExternalInput")
    dram["pb"] = nc.dram_tensor("pb", [BPC, 1, lqc], F32R, kind="ExternalInput")
    # small consts ride ONE DMA: [w1c(8) | w2c(4) | ident(128) | w3c(4) | obc(4)];
    # the f32r matmul views are made by one on-chip DVE copy
    dram["cpack"] = nc.dram_tensor("cpack", [128, 148], F32, kind="ExternalInput")
    dram["ow_t"] = nc.dram_tensor("ow_t", [4 * D, D], F32R, kind="ExternalInput")
    dram["out_t"] = nc.dram_tensor("out_t", [BPC, D, LC], F32, kind="ExternalOutput")
    if DEBUG_TAPS:
        dram["dbg_augr"] = nc.dram_tensor("dbg_augr", [2, lqc], F32R, kind="ExternalOutput")
        dram["dbg_rs"] = nc.dram_tensor("dbg_rs", [128, I_T], F32, kind="ExternalOutput")
        dram["dbg_irs"] = nc.dram_tensor("dbg_irs", [128, I_T], F32, kind="ExternalOutput")
        dram["dbg_c1"] = nc.dram_tensor("dbg_c1", [128, I_T], F32, kind="ExternalOutput")
        dram["dbg_zinv"] = nc.dram_tensor("dbg_zinv", [128, 3], F32, kind="ExternalOutput")
        dram["dbg_exp0"] = nc.dram_tensor("dbg_exp0", [128, lqc], BF16, kind="ExternalOutput")
        dram["dbg_expt0"] = nc.dram_tensor("dbg_expt0", [128, LC], BF16, kind="ExternalOutput")
        dram["dbg_z0"] = nc.dram_tensor("dbg_z0", [128, D], BF16, kind="ExternalOutput")

    with tile.TileContext(nc) as tc:
        with tc.tile_pool(name="sb", bufs=4) as sb, \
             tc.tile_pool(name="small", bufs=1) as small, \
             tc.tile_pool(name="consts", bufs=1) as cpool, \
             tc.tile_pool(name="psum", bufs=8, space="PSUM") as psum:
            rowps = None
            # ---- consts: two packed DMAs + an on-chip ones tile ----
            cpf = cpool.tile([128, 148], F32)
            nc.sync.dma_start(out=cpf[:], in_=dram["cpack"].ap())
            cpr = cpool.tile([128, 140], F32R)
            nc.vector.tensor_copy(cpr[:], cpf[:, 0:140])
            w1c, w2cb, identb = cpr[:, 0:8], cpr[:, 8:12], cpr[:, 12:140]
            w3c, obc = cpf[:, 140:144], cpf[:, 144:148]
            ones_f = cpool.tile([2, 128], F32)
            nc.vector.memset(ones_f[:], 1.0)
            ones2 = cpool.tile([2, 128], F32R)
            nc.vector.tensor_copy(ones2[:], ones_f[:])
            ow = []
            for f in range(F_T):
                t = cpool.tile([128, D], F32R, tag="ow", bufs=F_T, name=f"ow{f}")
                ow.append(t)
            consts = (ones2, w1c, w2cb, identb, w3c, ow, obc)
            pools = (sb, small, psum, rowps)

            fr = _emit_front(nc, pools, consts, dram, 0, lqc)
            # ow is needed only at b0's out stage: DMA after b0's inputs
            for f in range(F_T):
                nc.sync.dma_start(out=ow[f][:],
                                  in_=dram["ow_t"].ap()[f * 128:(f + 1) * 128, :])
            _emit_mid(nc, pools, consts, dram, 0, lqc, fr)
            fr1 = _emit_front(nc, pools, consts, dram, 1, lqc)
            _emit_back(nc, pools, consts, dram, 0, lqc, fr)
            _emit_mid(nc, pools, consts, dram, 1, lqc, fr1)
            _emit_back(nc, pools, consts, dram, 1, lqc, fr1)

    split_multi_waits(nc)
    return nc


_NC_CACHE = {}
_LAST_LQC = None


def _get_nc(lqc=None):
    global _LAST_LQC
    if lqc is None:
        lqc = _LAST_LQC if _LAST_LQC is not None else 288
    if lqc not in _NC_CACHE:
        _NC_CACHE[lqc] = build(lqc)
    _LAST_LQC = lqc
    return _NC_CACHE[lqc]


def make_in_maps(C, Q, cmask, qmask, w, out_w, out_b, lqc):
    C = np.asarray(C, dtype=np.float32)
    Q = np.asarray(Q, dtype=np.float32)
    cmask = np.asarray(cmask, dtype=np.float32)
    qmask = np.asarray(qmask, dtype=np.float32)
    w = np.asarray(w, dtype=np.float32)
    out_w = np.asarray(out_w, dtype=np.float32)
    out_b = np.asarray(out_b, dtype=np.float32)

    # packed consts; w1c holds duplicated column pairs (fp32r matmul
    # output free must be >= 2)
    cpack = np.zeros((128, 148), dtype=np.float32)
    cpack[:, 0:8] = np.repeat(w[:D].reshape(K_T, 128).T, 2, axis=1)
    cpack[:, 8:12] = w[D:2 * D].reshape(K_T, 128).T
    cpack[:, 12:140] = np.eye(128, dtype=np.float32)
    cpack[:, 140:144] = w[2 * D:].reshape(K_T, 128).T
    cpack[:, 144:148] = out_b.reshape(K_T, 128).T
    ow_t = np.ascontiguousarray(out_w.T)

    in_maps = []
    for c in range(NCORES):
        gsl = slice(c * BPC, (c + 1) * BPC)
        import ml_dtypes
        bf16 = ml_dtypes.bfloat16
        q_t = np.zeros((BPC, 128, K_T * lqc), dtype=np.float32)
        q_nat = np.zeros((BPC, lqc, D), dtype=np.float32)
        pb = np.zeros((BPC, 1, lqc), dtype=np.float32)
        for bi, g in enumerate(range(c * BPC, (c + 1) * BPC)):
            idx = np.nonzero(qmask[g] > 0.5)[0]
            cnt = len(idx)
            qct = np.zeros((D, lqc), dtype=np.float32)
            qct[:, :cnt] = Q[g][idx].T
            # pack the 4 k-tiles of Q^T side by side: [128, K_T*lqc]
            q_t[bi] = (
                qct.reshape(K_T, 128, lqc).transpose(1, 0, 2).reshape(128, K_T * lqc))
            q_nat[bi, :cnt, :] = Q[g][idx]
            pb[bi, 0, cnt:] = -MASK_BIAS
        cm_nat = cmask[gsl][:, :, None] * C[gsl]
        cm_col = np.repeat(cmask[gsl].reshape(BPC, I_T, 128).transpose(0, 2, 1),
                           2, axis=2)
        # quarter-packed C^T: c_t[b, q, p, k*256+c] = C^T[k*128+p, q*256+c]
        ctt = C[gsl].transpose(0, 2, 1).reshape(BPC, K_T, 128, 4, 256)
        c_t = np.ascontiguousarray(ctt.transpose(0, 3, 2, 1, 4).reshape(BPC, 4, 128, LC))
        in_maps.append({
            "c_t": c_t,
            "cm_nat": np.ascontiguousarray(cm_nat.astype(bf16)),
            "cm_col": np.ascontiguousarray(cm_col.astype(bf16)),
            "q_t": q_t, "q_nat": q_nat, "pb": pb,
            "cpack": cpack, "ow_t": ow_t,
        })
    return in_maps


def kernel(C, Q, cmask, qmask, w, out_w, out_b):
    qmask = np.asarray(qmask, dtype=np.float32)
    maxcnt = int((qmask > 0.5).sum(axis=1).max())
    lqc = min(LQ, max(64, ((maxcnt + 31) // 32) * 32))
    nc = _get_nc(lqc)
    in_maps = make_in_maps(C, Q, cmask, qmask, w, out_w, out_b, lqc)
    res = run_bass_kernel_spmd(nc, in_maps, list(range(NCORES)))
    outs = [res.results[i]["out_t"].transpose(0, 2, 1) for i in range(NCORES)]
    return np.ascontiguousarray(np.concatenate(outs, axis=0))
